# revision 2
# baseline (speedup 1.0000x reference)
"""Causal multi-head attention (B=2, T=4096, D=1024, H=16, HD=64) on 8 trn2
NeuronCores — fused-pipeline version.

Sharding: core c handles batch b = c//4 and head group g = c%4 (heads
4g..4g+3).  Per core: qkv projection (bf16), causal flash-attention in
transposed (S^T) layout, per-tile normalization (in-SBUF ones-matmul
broadcast of 1/denominator), and a partial out-projection, all software-
pipelined per 512-query tile:

  iter j:  [dma xT(j+1)] [attention(j) + interleaved out-proj(j-1) groups]
           [proj(j+1)] [norm broadcast(j)]

Math notes (same as baseline):
  - v-bias folded into host epilogue (softmax rows sum to 1).
  - softmax without max subtraction (scores O(6); exp fits fp32/bf16).
  - denominators ride as a 65th ones-column in v.
dtypes: x/w_qkv/q/k/v/P bf16, wout f32r, accum fp32.
"""

import numpy as np

import concourse.bass as bass
import concourse.mybir as mybir
import concourse.tile as tile
from concourse import bacc
from concourse.bass_utils import run_bass_kernel_spmd
from concourse.masks import make_identity, make_upper_triangular

F32 = mybir.dt.float32
F32R = mybir.dt.float32r
BF16 = mybir.dt.bfloat16
AF = mybir.ActivationFunctionType
MUL = mybir.AluOpType.mult

B, D, H, HD = 2, 1024, 16, 64
SCALE = 1.0 / np.sqrt(HD)


def build(T=4096, reps=1):
    NJ = T // 512
    NT = T // 128
    DC = D // 128

    nc = bacc.Bacc("TRN2", target_bir_lowering=False, debug=False, num_devices=8)

    xt_d = nc.dram_tensor("xt", [D, T], BF16, kind="ExternalInput")
    wqk_d = nc.dram_tensor("wqk", [D, 512], BF16, kind="ExternalInput")
    wv_d = nc.dram_tensor("wv", [D, 256], BF16, kind="ExternalInput")
    bqk_d = nc.dram_tensor("bqk", [128, 4], F32, kind="ExternalInput")
    wout_d = nc.dram_tensor("wout", [128, 2, D], F32R, kind="ExternalInput")
    y_d = nc.dram_tensor("y", [T, D], F32, kind="ExternalOutput")

    with tile.TileContext(nc) as tc:
        with (
            tc.tile_pool(name="const", bufs=1) as cp,
            tc.tile_pool(name="persist", bufs=1) as pp,
        ):
            # ---------- constants / weights (outside the timing loop)
            tri32 = cp.tile([128, 128], F32, tag="tri32")
            make_upper_triangular(nc, tri32[:], val=1.0, diag=True)
            tri = cp.tile([128, 128], BF16, tag="tri")
            nc.vector.tensor_copy(tri[:], tri32[:])

            id32 = cp.tile([64, 128], F32, tag="id32")
            nc.vector.memset(id32[:], 0.0)
            make_identity(nc, id32[:, 64:128], nomemset=True)
            ident = cp.tile([64, 128], F32R, tag="ident")
            with nc.allow_low_precision(reason="identity matrix is exact in f32r"):
                nc.vector.tensor_copy(ident[:], id32[:])

            bqk_sb = cp.tile([128, 4], F32, tag="bqk")
            nc.sync.dma_start(bqk_sb[:], bqk_d[:])
            wqk_sb = cp.tile([128, DC, 512], BF16, tag="wqk")
            nc.sync.dma_start(wqk_sb[:], wqk_d.rearrange("(dc p) c -> p dc c", p=128))
            wv_sb = cp.tile([128, DC, 256], BF16, tag="wv")
            nc.sync.dma_start(wv_sb[:], wv_d.rearrange("(dc p) c -> p dc c", p=128))
            wout_sb = cp.tile([128, 2, D], F32R, tag="wout")
            nc.sync.dma_start(wout_sb[:], wout_d[:])

            # ---------- persistent state
            kT = pp.tile([128, 2, T], BF16, tag="kT")          # [qk-col, pair, t]
            v_sb = pp.tile([128, NT, 4, 65], BF16, tag="v")    # [t%128, tchunk, head, hd+one]
            nc.vector.memset(v_sb[:, :, :, 64:65], 1.0)

            def body():
                with (
                    tc.tile_pool(name="wx", bufs=2) as wx,
                    tc.tile_pool(name="wq", bufs=2) as wq,
                    tc.tile_pool(name="wt", bufs=3) as wt,
                    tc.tile_pool(name="wo", bufs=2) as wo,
                    tc.tile_pool(name="wr", bufs=4) as wr,
                    tc.tile_pool(name="wy", bufs=2) as wy,
                    tc.tile_pool(name="pss", bufs=2, space="PSUM") as ps_s,
                    tc.tile_pool(name="pso", bufs=1, space="PSUM") as ps_o,
                    tc.tile_pool(name="psm", bufs=2, space="PSUM") as ps_m,
                ):
                    def dma_xt(jj):
                        # per-dc DMAs so proj can start on slab 0 while the
                        # rest stream in (kills the body-start stall)
                        xT = wx.tile([128, DC, 512], BF16, tag="xT")
                        src = xt_d[:, 512 * jj : 512 * jj + 512].rearrange(
                            "(dc p) t -> p dc t", p=128
                        )
                        for dc in range(DC):
                            nc.sync.dma_start(xT[:, dc : dc + 1], src[:, dc : dc + 1])
                        return xT

                    def proj_groups(jj, xT):
                        """qkv projection for tile jj as a list of PE-work
                        closures (one accumulation group each)."""
                        qTj = wq.tile([128, 2, 512], BF16, tag="qTj")
                        t0 = 512 * jj
                        groups = []
                        for cc in range(4):
                            def gqk(cc=cc, qTj=qTj, xT=xT, t0=t0):
                                pqk = ps_m.tile([128, 512], F32, tag="m", name="pqk")
                                for dc in range(DC):
                                    nc.tensor.matmul(
                                        pqk[:],
                                        wqk_sb[:, dc, 128 * cc : 128 * (cc + 1)],
                                        xT[:, dc],
                                        start=(dc == 0),
                                        stop=(dc == DC - 1),
                                    )
                                dst = qTj[:, cc] if cc < 2 else kT[:, cc - 2, t0 : t0 + 512]
                                nc.vector.tensor_scalar_add(dst, pqk[:], bqk_sb[:, cc : cc + 1])
                            groups.append(gqk)
                        for ts in range(4):
                            def gv(ts=ts, jj=jj, xT=xT):
                                pv = ps_m.tile([128, 512], F32, tag="m", name="pv")
                                for dc in range(DC):
                                    nc.tensor.matmul(
                                        pv[:, 0:256],
                                        xT[:, dc, 128 * ts : 128 * (ts + 1)],
                                        wv_sb[:, dc],
                                        start=(dc == 0),
                                        stop=(dc == DC - 1),
                                    )
                                nc.vector.tensor_copy(
                                    v_sb[:, 4 * jj + ts, :, 0:64],
                                    pv[:, 0:256].rearrange("p (h c) -> p h c", h=4),
                                )
                            groups.append(gv)
                        return qTj, groups

                    def attention(j, qTj, pending):
                        """Causal attention for tile j; pops one pending
                        closure per chunk to fill PE slack.  Output oT2
                        [128, pair, 512]: partitions 0-63 = even head of the
                        pair (scaled), 64-127 = odd head (filled by the shift
                        closures returned alongside)."""
                        oT2 = wo.tile([128, 2, 512], F32R, tag="oT2")
                        shifts = []
                        nchunk = 4 * (j + 1)
                        for hp in range(2):
                            psO = [
                                ps_o.tile([65, 512], F32, tag=f"o{hh}", name=f"psO{hh}")
                                for hh in range(2)
                            ]
                            for i in range(nchunk):
                                dlt = max(0, 128 * i - 512 * j)
                                pS = ps_s.tile([128, 1024], F32, tag="pS")
                                for hh in range(2):
                                    nc.tensor.matmul(
                                        pS[:, 512 * hh + dlt : 512 * (hh + 1)],
                                        kT[64 * hh : 64 * (hh + 1), hp, 128 * i : 128 * (i + 1)],
                                        qTj[64 * hh : 64 * (hh + 1), hp, dlt:512],
                                        start=True,
                                        stop=True,
                                    )
                                pT = wt.tile([128, 2, 512], BF16, tag="pT")
                                pSv = pS[:].rearrange("p (h w) -> p h w", h=2)
                                nc.scalar.activation(
                                    pT[:, :, dlt:512], pSv[:, :, dlt:512], AF.Exp, scale=SCALE
                                )
                                if i >= 4 * j:  # diagonal block: causal 0/1 mask
                                    for hh in range(2):
                                        nc.vector.tensor_tensor(
                                            pT[:, hh, dlt : dlt + 128],
                                            pT[:, hh, dlt : dlt + 128],
                                            tri[:],
                                            MUL,
                                        )
                                for hh in range(2):
                                    nc.tensor.matmul(
                                        psO[hh][0:65, dlt:512],
                                        v_sb[:, i, 2 * hp + hh, :],
                                        pT[:, hh, dlt:512],
                                        start=(i == 0),
                                        stop=(i == nchunk - 1),
                                        skip_group_check=True,
                                    )
                                if pending:
                                    pending.pop(0)()
                            for hh in range(2):
                                h = 2 * hp + hh
                                rrow = wr.tile([1, 512], F32, tag="rrow", name=f"rr{h}")
                                nc.vector.reciprocal(rrow[:], psO[hh][64:65, :])
                                # broadcast 1/denom down partitions (Pool),
                                # then scaled drain (DVE)
                                rbc = wr.tile([64, 512], F32, tag="rbc", name=f"rb{h}")
                                nc.gpsimd.partition_broadcast(rbc[:], rrow[:])
                                if hh == 0:
                                    nc.vector.tensor_copy(
                                        oT2[0:64, hp, :], psO[hh][0:64, :]
                                    )
                                    nc.vector.tensor_tensor(
                                        oT2[0:64, hp, :],
                                        oT2[0:64, hp, :].bitcast(F32),
                                        rbc[:],
                                        MUL,
                                    )
                                else:
                                    # odd head: scale into tmp, later PE-shift
                                    # to partitions 64-127 of oT2
                                    tmp = wr.tile([64, 512], F32R, tag="tmp", name=f"tm{h}")
                                    nc.vector.tensor_copy(tmp[:], psO[hh][0:64, :])
                                    nc.vector.tensor_tensor(
                                        tmp[:], tmp[:].bitcast(F32), rbc[:], MUL
                                    )

                                    def shift(hp=hp, tmp=tmp):
                                        psh = ps_m.tile([128, 512], F32, tag="m", name="psh")
                                        nc.tensor.matmul(
                                            psh[:], ident[:], tmp[:],
                                            start=True, stop=True,
                                        )
                                        nc.vector.tensor_copy(
                                            oT2[64:128, hp, :], psh[64:128, :]
                                        )
                                    shifts.append(shift)
                        while pending:
                            pending.pop(0)()
                        return oT2, shifts

                    def outproj_groups(jp, oT):
                        groups = []
                        t0 = 512 * jp
                        for jt in range(4):
                            y_box = []
                            for nh in range(2):
                                def g(jt=jt, nh=nh, oT=oT, t0=t0, y_box=y_box):
                                    if nh == 0:
                                        y_box.append(
                                            wy.tile([128, 1024], F32, tag="y", name="ysb")
                                        )
                                    y_sb = y_box[0]
                                    pY = ps_m.tile([128, 512], F32, tag="m", name="pY")
                                    for p in range(2):
                                        nc.tensor.matmul(
                                            pY[:],
                                            oT[0:128, p, 128 * jt : 128 * (jt + 1)],
                                            wout_sb[:, p, 512 * nh : 512 * (nh + 1)],
                                            start=(p == 0),
                                            stop=(p == 1),
                                        )
                                    nc.vector.tensor_copy(
                                        y_sb[:, 512 * nh : 512 * (nh + 1)], pY[:]
                                    )
                                    if nh == 1:
                                        nc.sync.dma_start(
                                            y_d[t0 + 128 * jt : t0 + 128 * (jt + 1), :],
                                            y_sb[:],
                                        )
                                groups.append(g)
                        return groups

                    def interleave(a, b):
                        out = []
                        for x, y in zip(a, b):
                            out += [x, y]
                        longer = a if len(a) > len(b) else b
                        out += longer[min(len(a), len(b)):]
                        return out

                    # ---------------- the pipeline ----------------
                    xT = dma_xt(0)
                    qTj, pg = proj_groups(0, xT)
                    for g in pg:
                        g()
                    pending = []
                    for j in range(NJ):
                        if j + 1 < NJ:
                            xT = dma_xt(j + 1)
                        oT2, shifts = attention(j, qTj, pending)
                        if j + 1 < NJ:
                            qTj, pg = proj_groups(j + 1, xT)
                            for g in pg:
                                g()
                        pending = shifts + outproj_groups(j, oT2)
                    for g in pending:
                        g()

            if reps == 1:
                body()
            else:
                with tc.For_i(0, reps, 1):
                    body()

    nc.compile()
    return nc


def shard_inputs(x, w_qkv, b_qkv, w_out, T):
    """Build the 8 per-core input maps (core c: batch c//4, head group c%4)."""
    import ml_dtypes

    x = np.asarray(x, dtype=np.float32)
    w_qkv = np.asarray(w_qkv, dtype=np.float32)
    b_qkv = np.asarray(b_qkv, dtype=np.float32)
    w_out = np.asarray(w_out, dtype=np.float32)
    bf16 = ml_dtypes.bfloat16
    in_maps = []
    for c in range(8):
        b, g = c // 4, c % 4
        qcols = slice(4 * g * 64, (4 * g + 4) * 64)
        kcols = slice(D + 4 * g * 64, D + (4 * g + 4) * 64)
        vcols = slice(2 * D + 4 * g * 64, 2 * D + (4 * g + 4) * 64)
        wqk = np.concatenate([w_qkv[:, qcols], w_qkv[:, kcols]], axis=1)  # [D, 512]
        wv = np.ascontiguousarray(w_qkv[:, vcols])  # [D, 256]
        bqk = np.concatenate([b_qkv[qcols], b_qkv[kcols]]).reshape(4, 128).T  # [128,4]
        # [128, 2, D]: partition = (head-in-pair, hd), dim1 = pair
        wout = np.ascontiguousarray(
            w_out[256 * g : 256 * (g + 1), :]
            .reshape(2, 2, 64, D)
            .transpose(1, 2, 0, 3)
            .reshape(128, 2, D)
        )
        in_maps.append(
            {
                "xt": np.ascontiguousarray(x[b, :T].T).astype(bf16),
                "wqk": np.ascontiguousarray(wqk).astype(bf16),
                "wv": wv.astype(bf16),
                "bqk": np.ascontiguousarray(bqk),
                "wout": wout,
            }
        )
    return in_maps


def assemble_output(results, b_qkv, b_out, w_out, T):
    b_qkv = np.asarray(b_qkv, dtype=np.float32)
    b_out = np.asarray(b_out, dtype=np.float32)
    w_out = np.asarray(w_out, dtype=np.float32)
    extra = b_out + b_qkv[2 * D :] @ w_out  # v-bias folds through softmax
    y = np.zeros((B, T, D), dtype=np.float32)
    for c in range(8):
        y[c // 4] += results[c]["y"]
    y += extra[None, None, :]
    return y


_cache = {}


def kernel(x, w_qkv, b_qkv, w_out, b_out):
    x = np.asarray(x, dtype=np.float32)
    T = x.shape[1]
    if T not in _cache:
        _cache[T] = build(T=T, reps=1)
    nc = _cache[T]
    in_maps = shard_inputs(x, w_qkv, b_qkv, w_out, T)
    for _attempt in range(3):
        res = run_bass_kernel_spmd(nc, in_maps, core_ids=list(range(8)), trace=False)
        y = assemble_output(res.results, b_qkv, b_out, w_out, T)
        if np.isfinite(y).all():  # guard against transient device flakes
            return y
    return y
